# revision 8
# baseline (speedup 1.0000x reference)
"""Trainium2 Bass kernel for nn_DiffDMC (quad triangulation).

Strategy (sharding: quads split along Q across 8 cores):
  host:   gather qv = verts[quads] (random-access gather, numpy),
          pad + shard along Q across the 8 NeuronCores
  device: per [128 x F] tile of quads -- 6 edge squared-lengths, 12
          interior-angle cosines via law of cosines, concavity via the
          Lagrange identity dot(cross(a,b),cross(c,d)) = (ac)(bd)-(ad)(bc)
          (all pair-dots derived from edge sumsqs), group in {0..3},
          decision-risk flag, and the [Q,6] triangle-index output
  host:   re-resolve risk-flagged quads (|margin| < 1e-4) with an exact
          eager-jnp-on-CPU mirror of the reference decision math, then
          stable-partition by group and emit faces [2Q, 3]

Outputs a tuple (verts, faces) matching the reference.
"""
import sys
for _p in ("/opt/trn_rl_repo", "/root/.axon_site/_ro/trn_rl_repo"):
    if _p not in sys.path:
        sys.path.append(_p)

import numpy as np

P = 128
NCORES = 8
Q_TOTAL = 2_000_000
N_SHARD = Q_TOTAL // NCORES            # 250_000
CHUNK_F = [256] * 7 + [162]            # per-core tile free-dims
N_PAD = P * sum(CHUNK_F)               # 250_112

DELTA = 1e-4
NORM_BIAS = 2e-24

EDGES = [(0, 1), (0, 2), (0, 3), (1, 2), (1, 3), (2, 3)]
EIX = {}
for _i, (_a, _b) in enumerate(EDGES):
    EIX[(_a, _b)] = _i
    EIX[(_b, _a)] = _i
TRIS = [(0, 1, 3), (1, 2, 3), (0, 1, 2), (0, 2, 3)]
IDX1 = np.array([0, 1, 3, 1, 2, 3])
IDX2 = np.array([0, 1, 2, 0, 2, 3])

_NC_CACHE = {}
LAST_EXEC_TIME_NS = None


def _build_nc():
    import concourse.bacc as bacc
    from concourse import mybir
    from concourse.tile import TileContext

    F32 = mybir.dt.float32
    I32 = mybir.dt.int32
    ALU = mybir.AluOpType
    ACTF = mybir.ActivationFunctionType

    nc = bacc.Bacc("TRN2", target_bir_lowering=False, debug=False,
                   enable_asserts=True, num_devices=NCORES)
    qv_d = nc.dram_tensor("qv", [N_PAD, 12], F32, kind="ExternalInput")
    quads_d = nc.dram_tensor("quads", [N_PAD, 4], I32, kind="ExternalInput")
    tri_d = nc.dram_tensor("tri", [N_PAD, 6], I32, kind="ExternalOutput")
    grp_d = nc.dram_tensor("grp", [N_PAD], F32, kind="ExternalOutput")

    qv_ap = qv_d.ap()
    quads_ap = quads_d.ap()
    tri_ap = tri_d.ap()
    grp_ap = grp_d.ap().rearrange("(n one) -> n one", one=1)

    def emit_chunk(pio, pw, pt, c0, F, bias_t):
        TT = nc.vector.tensor_tensor
        TS = nc.vector.tensor_scalar
        ACT = nc.scalar.activation

        def tile(tag, pool=pw, dt=F32, f=None):
            return pool.tile([P, f if f is not None else F], dt, tag=tag, name=tag)

        qt = tile("qt", pio, I32, 4 * F)
        nc.sync.dma_start(qt[:], quads_ap[c0:c0 + P * F, :].rearrange(
            "(p f) c -> p (f c)", p=P))
        vt = tile("vt", pio, F32, 12 * F)
        nc.sync.dma_start(vt[:], qv_ap[c0:c0 + P * F, :].rearrange(
            "(p f) c -> p (f c)", p=P))

        comp = lambda j, c: vt[:, 3 * j + c::12]

        s = []
        r = []
        for k, (a, b) in enumerate(EDGES):
            sq = []
            for c in range(3):
                d = tile(f"tmp_d{c}", pt)
                TT(out=d[:], in0=comp(a, c), in1=comp(b, c), op=ALU.subtract)
                q = tile(f"tmp_q{c}", pt)
                ACT(q[:], d[:], ACTF.Square)
                sq.append(q)
            sk = tile(f"s{k}")
            TT(out=sk[:], in0=sq[0][:], in1=sq[1][:], op=ALU.add)
            TT(out=sk[:], in0=sk[:], in1=sq[2][:], op=ALU.add)
            t = tile("tmp_t", pt)
            ACT(t[:], sk[:], ACTF.Sqrt, bias=bias_t[:], scale=2.0)
            rk = tile(f"r{k}")
            nc.vector.reciprocal_approx_fast(rk[:], t[:])
            s.append(sk)
            r.append(rk)

        nums = {}
        tmax = []
        for ti, (a, b, c) in enumerate(TRIS):
            A, B, C = s[EIX[(a, b)]], s[EIX[(a, c)]], s[EIX[(b, c)]]
            rab, rac, rbc = r[EIX[(a, b)]], r[EIX[(a, c)]], r[EIX[(b, c)]]
            cos = {}
            for (v, e1, e2, e3, r1, r2) in (
                (a, A, B, C, rab, rac),
                (b, A, C, B, rab, rbc),
                (c, B, C, A, rac, rbc),
            ):
                n = tile(f"num{ti}_{v}")
                TT(out=n[:], in0=e1[:], in1=e2[:], op=ALU.add)
                TT(out=n[:], in0=n[:], in1=e3[:], op=ALU.subtract)
                nums[(ti, v)] = n
                p_ = tile("tmp_p", pt)
                nc.gpsimd.tensor_tensor(out=p_[:], in0=r1[:], in1=r2[:], op=ALU.mult)
                cs = tile(f"tmp_cos{len(cos)}", pt)
                TT(out=cs[:], in0=n[:], in1=p_[:], op=ALU.mult)
                cos[v] = cs
            cv = list(cos.values())
            m = tile(f"tmax{ti}")
            TT(out=m[:], in0=cv[0][:], in1=cv[1][:], op=ALU.max)
            TT(out=m[:], in0=m[:], in1=cv[2][:], op=ALU.max)
            tmax.append(m)

        a1 = tile("a1")
        TT(out=a1[:], in0=tmax[0][:], in1=tmax[1][:], op=ALU.max)
        a2 = tile("a2")
        TT(out=a2[:], in0=tmax[2][:], in1=tmax[3][:], op=ALU.max)
        ge = tile("ge")
        TT(out=ge[:], in0=a1[:], in1=a2[:], op=ALU.is_ge)
        adiff = tile("tmp_ad", pt)
        TT(out=adiff[:], in0=a1[:], in1=a2[:], op=ALU.subtract)
        aabs = tile("tmp_aabs", pt)
        ACT(aabs[:], adiff[:], ACTF.Abs)
        risk = tile("risk")
        TS(out=risk[:], in0=aabs[:], scalar1=DELTA, scalar2=None, op0=ALU.is_lt)

        N1, N2, N3 = nums[(2, 0)], nums[(3, 0)], nums[(0, 0)]
        A2, B2, C2, D2 = nums[(2, 1)], nums[(0, 3)], nums[(0, 1)], nums[(1, 1)]

        def risky_sub(x, y, name):
            dd = tile(name)
            TT(out=dd[:], in0=x[:], in1=y[:], op=ALU.subtract)
            ax = tile("tmp_ax", pt)
            ACT(ax[:], x[:], ACTF.Abs)
            ay = tile("tmp_ay", pt)
            ACT(ay[:], y[:], ACTF.Abs)
            TT(out=ax[:], in0=ax[:], in1=ay[:], op=ALU.add)
            TS(out=ax[:], in0=ax[:], scalar1=DELTA, scalar2=None, op0=ALU.mult)
            ad = tile("tmp_adm", pt)
            ACT(ad[:], dd[:], ACTF.Abs)
            rr = tile("tmp_rr", pt)
            TT(out=rr[:], in0=ad[:], in1=ax[:], op=ALU.is_lt)
            TT(out=risk[:], in0=risk[:], in1=rr[:], op=ALU.max)
            return dd

        t1 = tile("tmp_t1", pt)
        nc.gpsimd.tensor_tensor(out=t1[:], in0=N1[:], in1=N2[:], op=ALU.mult)
        t2 = tile("tmp_t2", pt)
        nc.gpsimd.tensor_tensor(out=t2[:], in0=N3[:], in1=s[EIX[(0, 2)]][:], op=ALU.mult)
        TT(out=t2[:], in0=t2[:], in1=t2[:], op=ALU.add)
        d4a = risky_sub(t1, t2, "d4a")

        t3 = tile("tmp_t3", pt)
        nc.gpsimd.tensor_tensor(out=t3[:], in0=D2[:], in1=A2[:], op=ALU.subtract)
        nc.gpsimd.tensor_tensor(out=t3[:], in0=C2[:], in1=t3[:], op=ALU.mult)
        t4 = tile("tmp_t4", pt)
        nc.gpsimd.tensor_tensor(out=t4[:], in0=A2[:], in1=B2[:], op=ALU.mult)
        d4b = risky_sub(t3, t4, "d4b")

        dmin = tile("tmp_dmin", pt)
        TT(out=dmin[:], in0=d4a[:], in1=d4b[:], op=ALU.min)
        conc = tile("conc")
        TS(out=conc[:], in0=dmin[:], scalar1=0.0, scalar2=None, op0=ALU.is_lt)

        omg = tile("omg")
        TS(out=omg[:], in0=ge[:], scalar1=-1.0, scalar2=1.0, op0=ALU.mult, op1=ALU.add)
        tpg = tile("tmp_tpg", pt)
        TS(out=tpg[:], in0=ge[:], scalar1=2.0, scalar2=None, op0=ALU.add)
        omc = tile("omc")
        TS(out=omc[:], in0=conc[:], scalar1=-1.0, scalar2=1.0, op0=ALU.mult, op1=ALU.add)

        gout = tile("gout", pio)
        TT(out=gout[:], in0=conc[:], in1=omg[:], op=ALU.mult)
        t5 = tile("tmp_t5", pt)
        TT(out=t5[:], in0=omc[:], in1=tpg[:], op=ALU.mult)
        TT(out=gout[:], in0=gout[:], in1=t5[:], op=ALU.add)
        t6 = tile("tmp_t6", pt)
        TS(out=t6[:], in0=risk[:], scalar1=4.0, scalar2=None, op0=ALU.mult)
        TT(out=gout[:], in0=gout[:], in1=t6[:], op=ALU.add)

        usel = tile("usel")
        TT(out=usel[:], in0=conc[:], in1=ge[:], op=ALU.mult)
        t7 = tile("tmp_t7", pt)
        TT(out=t7[:], in0=omc[:], in1=omg[:], op=ALU.mult)
        TT(out=usel[:], in0=usel[:], in1=t7[:], op=ALU.add)

        qf = []
        for k in range(4):
            qk = tile(f"qf{k}")
            nc.scalar.copy(out=qk[:], in_=qt[:, k::4])
            qf.append(qk)
        s1f = tile("tmp_s1f", pt)
        TT(out=s1f[:], in0=qf[3][:], in1=qf[2][:], op=ALU.subtract)
        TT(out=s1f[:], in0=s1f[:], in1=usel[:], op=ALU.mult)
        TT(out=s1f[:], in0=s1f[:], in1=qf[2][:], op=ALU.add)
        s2f = tile("tmp_s2f", pt)
        TT(out=s2f[:], in0=qf[1][:], in1=qf[0][:], op=ALU.subtract)
        TT(out=s2f[:], in0=s2f[:], in1=usel[:], op=ALU.mult)
        TT(out=s2f[:], in0=s2f[:], in1=qf[0][:], op=ALU.add)

        ot = tile("ot", pio, I32, 6 * F)
        nc.gpsimd.tensor_copy(out=ot[:, 0::6], in_=qt[:, 0::4])
        nc.gpsimd.tensor_copy(out=ot[:, 1::6], in_=qt[:, 1::4])
        nc.vector.tensor_copy(out=ot[:, 2::6], in_=s1f[:])
        nc.vector.tensor_copy(out=ot[:, 3::6], in_=s2f[:])
        nc.gpsimd.tensor_copy(out=ot[:, 4::6], in_=qt[:, 2::4])
        nc.gpsimd.tensor_copy(out=ot[:, 5::6], in_=qt[:, 3::4])

        nc.sync.dma_start(
            tri_ap[c0:c0 + P * F, :].rearrange("(p f) c -> p (f c)", p=P), ot[:])
        nc.sync.dma_start(
            grp_ap[c0:c0 + P * F, :].rearrange("(p f) one -> p (f one)", p=P), gout[:])

    with TileContext(nc) as tc:
        with (
            tc.tile_pool(name="const", bufs=1) as pc,
            tc.tile_pool(name="io", bufs=2) as pio,
            tc.tile_pool(name="work", bufs=1) as pw,
            tc.tile_pool(name="tmp", bufs=2) as pt,
        ):
            bias_t = pc.tile([P, 1], F32, tag="bias", name="bias")
            nc.gpsimd.memset(bias_t[:], NORM_BIAS)
            c0 = 0
            for F in CHUNK_F:
                emit_chunk(pio, pw, pt, c0, F, bias_t)
                c0 += P * F
    nc.compile()
    return nc


def _get_nc():
    if "nc" not in _NC_CACHE:
        _NC_CACHE["nc"] = _build_nc()
    return _NC_CACHE["nc"]


def _ref_decisions_cpu(verts, quads_subset):
    """Eager jnp on CPU — bit-exact mirror of the reference decision math."""
    import jax
    import jax.numpy as jnp

    cpu = jax.devices("cpu")[0]
    with jax.default_device(cpu):
        verts_j = jnp.asarray(verts)
        quads_j = jnp.asarray(quads_subset)
        EPS = 1e-12

        def normalize(x):
            n = jnp.linalg.norm(x, axis=-1, keepdims=True)
            return x / jnp.maximum(n, EPS)

        def max_cos(a, b, c):
            c1 = jnp.sum(normalize(b - a) * normalize(c - a), axis=-1)
            c2 = jnp.sum(normalize(c - b) * normalize(a - b), axis=-1)
            c3 = jnp.sum(normalize(a - c) * normalize(b - c), axis=-1)
            return jnp.maximum(jnp.maximum(c1, c2), c3)

        qv = verts_j[quads_j]
        v0, v1, v2, v3 = qv[:, 0], qv[:, 1], qv[:, 2], qv[:, 3]
        n1_c1 = jnp.cross(v1 - v0, v2 - v0)
        n2_c1 = jnp.cross(v2 - v0, v3 - v0)
        n1_c2 = jnp.cross(v1 - v0, v3 - v0)
        n2_c2 = jnp.cross(v2 - v1, v3 - v1)
        is_concave = (jnp.sum(n1_c1 * n2_c1, axis=-1) < 0) | \
                     (jnp.sum(n1_c2 * n2_c2, axis=-1) < 0)
        angles1 = jnp.maximum(max_cos(v0, v1, v3), max_cos(v1, v2, v3))
        angles2 = jnp.maximum(max_cos(v0, v1, v2), max_cos(v0, v2, v3))
        ge = angles1 >= angles2
        use_cfg1 = jnp.where(is_concave, ge, ~ge)
        group = jnp.where(is_concave,
                          jnp.where(ge, 0, 1),
                          jnp.where(ge, 3, 2)).astype(jnp.int32)
        return np.asarray(group), np.asarray(use_cfg1)


def _ensure_ntff_hook():
    """Register the axon NTFF profiling hook (the minimal agent image lacks
    antenv.axon_hooks, so trn_boot degrades silently; re-wire it here)."""
    import types

    if "antenv.axon_hooks" not in sys.modules:
        mod = types.ModuleType("antenv.axon_hooks")
        mod._hook = None
        mod.set_axon_ntff_profile_hook = lambda h: setattr(mod, "_hook", h)
        mod.get_axon_ntff_profile_hook = lambda: mod._hook
        sys.modules["antenv.axon_hooks"] = mod
        try:
            import antenv
            antenv.axon_hooks = mod
        except Exception:
            pass
    mod = sys.modules["antenv.axon_hooks"]
    if mod.get_axon_ntff_profile_hook() is None:
        try:
            if "/root/.axon_site" not in sys.path:
                sys.path.append("/root/.axon_site")
            from trn_agent_boot import trn_boot
            hook = trn_boot._ntff_profile_via_ctypes("/opt/axon/libaxon_pjrt.so")
            if hook is not None:
                mod.set_axon_ntff_profile_hook(hook)
        except Exception:
            pass


def kernel(verts, quads, profile=False):
    global LAST_EXEC_TIME_NS
    from concourse import bass_utils

    verts = np.ascontiguousarray(np.asarray(verts, dtype=np.float32))
    quads = np.ascontiguousarray(np.asarray(quads, dtype=np.int32))
    Q = quads.shape[0]
    assert Q == Q_TOTAL, f"kernel hardcoded for Q={Q_TOTAL}, got {Q}"

    # host gather (random access) + shard
    qv = verts[quads.ravel()].reshape(-1, 12)   # [Q, 12] float32

    in_maps = []
    pad = N_PAD - N_SHARD
    for c in range(NCORES):
        lo = c * N_SHARD
        hi = lo + N_SHARD
        q_sh = np.concatenate([quads[lo:hi], np.repeat(quads[hi - 1:hi], pad, 0)])
        v_sh = np.concatenate([qv[lo:hi], np.repeat(qv[hi - 1:hi], pad, 0)])
        in_maps.append({"qv": np.ascontiguousarray(v_sh),
                        "quads": np.ascontiguousarray(q_sh)})

    nc = _get_nc()
    res = None
    if profile:
        try:
            _ensure_ntff_hook()
            bass_utils.upload_artifacts = lambda tmpdir: tmpdir  # no bucket here
            res = bass_utils.run_bass_kernel_spmd(
                nc, in_maps, core_ids=list(range(NCORES)), trace=True)
            LAST_EXEC_TIME_NS = res.exec_time_ns
        except Exception as e:
            print(f"profiled run failed ({e!r}); falling back to untraced run")
            res = None
    if res is None:
        res = bass_utils.run_bass_kernel_spmd(
            nc, in_maps, core_ids=list(range(NCORES)))

    tri = np.concatenate([res.results[c]["tri"][:N_SHARD] for c in range(NCORES)])
    grp = np.concatenate([res.results[c]["grp"][:N_SHARD] for c in range(NCORES)])

    gi = grp.astype(np.int32)
    risk = gi >= 4
    group = gi & 3

    # resolve risk-flagged quads with the exact reference decision math (CPU)
    ridx = np.flatnonzero(risk)
    if len(ridx):
        g_fix, u_fix = _ref_decisions_cpu(verts, quads[ridx])
        group[ridx] = g_fix
        qr = quads[ridx]
        tri[ridx, 2] = np.where(u_fix, qr[:, 3], qr[:, 2])
        tri[ridx, 3] = np.where(u_fix, qr[:, 1], qr[:, 0])

    order = np.concatenate([np.flatnonzero(group == g) for g in range(4)])
    faces = tri[order].reshape(-1, 3)
    return verts, faces


# revision 9
# speedup vs baseline: 1.0412x; 1.0412x over previous
"""Trainium2 Bass kernel for nn_DiffDMC (quad triangulation).

Strategy (sharding: quads split along Q across 8 cores):
  host:   gather qv = verts[quads] (random-access gather, numpy),
          pad + shard along Q across the 8 NeuronCores
  device: per [128 x F] tile of quads -- 6 edge squared-lengths, 12
          interior-angle cosines via law of cosines, concavity via the
          Lagrange identity dot(cross(a,b),cross(c,d)) = (ac)(bd)-(ad)(bc)
          (all pair-dots derived from edge sumsqs), group in {0..3},
          decision-risk flag, and the [Q,6] triangle-index output
  host:   re-resolve risk-flagged quads (|margin| < 1e-4) with an exact
          eager-jnp-on-CPU mirror of the reference decision math, then
          stable-partition by group and emit faces [2Q, 3]

Outputs a tuple (verts, faces) matching the reference.
"""
import sys
for _p in ("/opt/trn_rl_repo", "/root/.axon_site/_ro/trn_rl_repo"):
    if _p not in sys.path:
        sys.path.append(_p)

import numpy as np

P = 128
NCORES = 8
Q_TOTAL = 2_000_000
N_SHARD = Q_TOTAL // NCORES            # 250_000
CHUNK_F = [256] * 7 + [162]            # per-core tile free-dims
N_PAD = P * sum(CHUNK_F)               # 250_112

DELTA = 1e-4
NORM_BIAS = 2e-24

EDGES = [(0, 1), (0, 2), (0, 3), (1, 2), (1, 3), (2, 3)]
EIX = {}
for _i, (_a, _b) in enumerate(EDGES):
    EIX[(_a, _b)] = _i
    EIX[(_b, _a)] = _i
TRIS = [(0, 1, 3), (1, 2, 3), (0, 1, 2), (0, 2, 3)]
IDX1 = np.array([0, 1, 3, 1, 2, 3])
IDX2 = np.array([0, 1, 2, 0, 2, 3])

_NC_CACHE = {}
LAST_EXEC_TIME_NS = None


def _build_nc():
    import concourse.bacc as bacc
    from concourse import mybir
    from concourse.tile import TileContext

    F32 = mybir.dt.float32
    I32 = mybir.dt.int32
    ALU = mybir.AluOpType
    ACTF = mybir.ActivationFunctionType

    nc = bacc.Bacc("TRN2", target_bir_lowering=False, debug=False,
                   enable_asserts=True, num_devices=NCORES)
    qv_d = nc.dram_tensor("qv", [N_PAD, 12], F32, kind="ExternalInput")
    quads_d = nc.dram_tensor("quads", [N_PAD, 4], I32, kind="ExternalInput")
    tri_d = nc.dram_tensor("tri", [N_PAD, 6], I32, kind="ExternalOutput")
    grp_d = nc.dram_tensor("grp", [N_PAD], F32, kind="ExternalOutput")

    qv_ap = qv_d.ap()
    quads_ap = quads_d.ap()
    tri_ap = tri_d.ap()
    grp_ap = grp_d.ap().rearrange("(n one) -> n one", one=1)

    def emit_chunk(pio, pw, pt, c0, F, bias_t):
        TT = nc.vector.tensor_tensor
        TS = nc.vector.tensor_scalar
        ACT = nc.scalar.activation

        def tile(tag, pool=pw, dt=F32, f=None):
            return pool.tile([P, f if f is not None else F], dt, tag=tag, name=tag)

        qt = tile("qt", pio, I32, 4 * F)
        nc.sync.dma_start(qt[:], quads_ap[c0:c0 + P * F, :].rearrange(
            "(p f) c -> p (f c)", p=P))
        vt = tile("vt", pio, F32, 12 * F)
        nc.sync.dma_start(vt[:], qv_ap[c0:c0 + P * F, :].rearrange(
            "(p f) c -> p (f c)", p=P))

        comp = lambda j, c: vt[:, 3 * j + c::12]

        s = []
        r = []
        for k, (a, b) in enumerate(EDGES):
            sq = []
            for c in range(3):
                d = tile(f"tmp_d{c}", pt)
                TT(out=d[:], in0=comp(a, c), in1=comp(b, c), op=ALU.subtract)
                q = tile(f"tmp_q{c}", pt)
                ACT(q[:], d[:], ACTF.Square)
                sq.append(q)
            sk = tile(f"s{k}")
            TT(out=sk[:], in0=sq[0][:], in1=sq[1][:], op=ALU.add)
            TT(out=sk[:], in0=sk[:], in1=sq[2][:], op=ALU.add)
            t = tile("tmp_t", pt)
            ACT(t[:], sk[:], ACTF.Sqrt, bias=bias_t[:], scale=2.0)
            rk = tile(f"r{k}")
            nc.vector.reciprocal_approx_fast(rk[:], t[:])
            s.append(sk)
            r.append(rk)

        nums = {}
        tmax = []
        for ti, (a, b, c) in enumerate(TRIS):
            A, B, C = s[EIX[(a, b)]], s[EIX[(a, c)]], s[EIX[(b, c)]]
            rab, rac, rbc = r[EIX[(a, b)]], r[EIX[(a, c)]], r[EIX[(b, c)]]
            cos = {}
            for (v, e1, e2, e3, r1, r2) in (
                (a, A, B, C, rab, rac),
                (b, A, C, B, rab, rbc),
                (c, B, C, A, rac, rbc),
            ):
                n = tile(f"num{ti}_{v}")
                TT(out=n[:], in0=e1[:], in1=e2[:], op=ALU.add)
                TT(out=n[:], in0=n[:], in1=e3[:], op=ALU.subtract)
                nums[(ti, v)] = n
                p_ = tile("tmp_p", pt)
                TT(out=p_[:], in0=r1[:], in1=r2[:], op=ALU.mult)
                cs = tile(f"tmp_cos{len(cos)}", pt)
                TT(out=cs[:], in0=n[:], in1=p_[:], op=ALU.mult)
                cos[v] = cs
            cv = list(cos.values())
            m = tile(f"tmax{ti}")
            TT(out=m[:], in0=cv[0][:], in1=cv[1][:], op=ALU.max)
            TT(out=m[:], in0=m[:], in1=cv[2][:], op=ALU.max)
            tmax.append(m)

        a1 = tile("a1")
        TT(out=a1[:], in0=tmax[0][:], in1=tmax[1][:], op=ALU.max)
        a2 = tile("a2")
        TT(out=a2[:], in0=tmax[2][:], in1=tmax[3][:], op=ALU.max)
        ge = tile("ge")
        TT(out=ge[:], in0=a1[:], in1=a2[:], op=ALU.is_ge)
        adiff = tile("tmp_ad", pt)
        TT(out=adiff[:], in0=a1[:], in1=a2[:], op=ALU.subtract)
        aabs = tile("tmp_aabs", pt)
        ACT(aabs[:], adiff[:], ACTF.Abs)
        risk = tile("risk")
        TS(out=risk[:], in0=aabs[:], scalar1=DELTA, scalar2=None, op0=ALU.is_lt)

        N1, N2, N3 = nums[(2, 0)], nums[(3, 0)], nums[(0, 0)]
        A2, B2, C2, D2 = nums[(2, 1)], nums[(0, 3)], nums[(0, 1)], nums[(1, 1)]

        def risky_sub(x, y, name):
            dd = tile(name)
            TT(out=dd[:], in0=x[:], in1=y[:], op=ALU.subtract)
            ax = tile("tmp_ax", pt)
            ACT(ax[:], x[:], ACTF.Abs)
            ay = tile("tmp_ay", pt)
            ACT(ay[:], y[:], ACTF.Abs)
            TT(out=ax[:], in0=ax[:], in1=ay[:], op=ALU.add)
            TS(out=ax[:], in0=ax[:], scalar1=DELTA, scalar2=None, op0=ALU.mult)
            ad = tile("tmp_adm", pt)
            ACT(ad[:], dd[:], ACTF.Abs)
            rr = tile("tmp_rr", pt)
            TT(out=rr[:], in0=ad[:], in1=ax[:], op=ALU.is_lt)
            TT(out=risk[:], in0=risk[:], in1=rr[:], op=ALU.max)
            return dd

        t1 = tile("tmp_t1", pt)
        TT(out=t1[:], in0=N1[:], in1=N2[:], op=ALU.mult)
        t2 = tile("tmp_t2", pt)
        TT(out=t2[:], in0=N3[:], in1=s[EIX[(0, 2)]][:], op=ALU.mult)
        TT(out=t2[:], in0=t2[:], in1=t2[:], op=ALU.add)
        d4a = risky_sub(t1, t2, "d4a")

        t3 = tile("tmp_t3", pt)
        TT(out=t3[:], in0=D2[:], in1=A2[:], op=ALU.subtract)
        TT(out=t3[:], in0=C2[:], in1=t3[:], op=ALU.mult)
        t4 = tile("tmp_t4", pt)
        TT(out=t4[:], in0=A2[:], in1=B2[:], op=ALU.mult)
        d4b = risky_sub(t3, t4, "d4b")

        dmin = tile("tmp_dmin", pt)
        TT(out=dmin[:], in0=d4a[:], in1=d4b[:], op=ALU.min)
        conc = tile("conc")
        TS(out=conc[:], in0=dmin[:], scalar1=0.0, scalar2=None, op0=ALU.is_lt)

        omg = tile("omg")
        TS(out=omg[:], in0=ge[:], scalar1=-1.0, scalar2=1.0, op0=ALU.mult, op1=ALU.add)
        tpg = tile("tmp_tpg", pt)
        TS(out=tpg[:], in0=ge[:], scalar1=2.0, scalar2=None, op0=ALU.add)
        omc = tile("omc")
        TS(out=omc[:], in0=conc[:], scalar1=-1.0, scalar2=1.0, op0=ALU.mult, op1=ALU.add)

        gout = tile("gout", pio)
        TT(out=gout[:], in0=conc[:], in1=omg[:], op=ALU.mult)
        t5 = tile("tmp_t5", pt)
        TT(out=t5[:], in0=omc[:], in1=tpg[:], op=ALU.mult)
        TT(out=gout[:], in0=gout[:], in1=t5[:], op=ALU.add)
        t6 = tile("tmp_t6", pt)
        TS(out=t6[:], in0=risk[:], scalar1=4.0, scalar2=None, op0=ALU.mult)
        TT(out=gout[:], in0=gout[:], in1=t6[:], op=ALU.add)

        usel = tile("usel")
        TT(out=usel[:], in0=conc[:], in1=ge[:], op=ALU.mult)
        t7 = tile("tmp_t7", pt)
        TT(out=t7[:], in0=omc[:], in1=omg[:], op=ALU.mult)
        TT(out=usel[:], in0=usel[:], in1=t7[:], op=ALU.add)

        qf = []
        for k in range(4):
            qk = tile(f"qf{k}")
            nc.vector.tensor_copy(out=qk[:], in_=qt[:, k::4])
            qf.append(qk)
        s1f = tile("tmp_s1f", pt)
        TT(out=s1f[:], in0=qf[3][:], in1=qf[2][:], op=ALU.subtract)
        TT(out=s1f[:], in0=s1f[:], in1=usel[:], op=ALU.mult)
        TT(out=s1f[:], in0=s1f[:], in1=qf[2][:], op=ALU.add)
        s2f = tile("tmp_s2f", pt)
        TT(out=s2f[:], in0=qf[1][:], in1=qf[0][:], op=ALU.subtract)
        TT(out=s2f[:], in0=s2f[:], in1=usel[:], op=ALU.mult)
        TT(out=s2f[:], in0=s2f[:], in1=qf[0][:], op=ALU.add)

        ot = tile("ot", pio, I32, 6 * F)
        nc.vector.tensor_copy(out=ot[:, 0::6], in_=qt[:, 0::4])
        nc.vector.tensor_copy(out=ot[:, 1::6], in_=qt[:, 1::4])
        nc.vector.tensor_copy(out=ot[:, 2::6], in_=s1f[:])
        nc.vector.tensor_copy(out=ot[:, 3::6], in_=s2f[:])
        nc.vector.tensor_copy(out=ot[:, 4::6], in_=qt[:, 2::4])
        nc.vector.tensor_copy(out=ot[:, 5::6], in_=qt[:, 3::4])

        nc.sync.dma_start(
            tri_ap[c0:c0 + P * F, :].rearrange("(p f) c -> p (f c)", p=P), ot[:])
        nc.sync.dma_start(
            grp_ap[c0:c0 + P * F, :].rearrange("(p f) one -> p (f one)", p=P), gout[:])

    with TileContext(nc) as tc:
        with (
            tc.tile_pool(name="const", bufs=1) as pc,
            tc.tile_pool(name="io", bufs=2) as pio,
            tc.tile_pool(name="work", bufs=1) as pw,
            tc.tile_pool(name="tmp", bufs=2) as pt,
        ):
            bias_t = pc.tile([P, 1], F32, tag="bias", name="bias")
            nc.gpsimd.memset(bias_t[:], NORM_BIAS)
            c0 = 0
            for F in CHUNK_F:
                emit_chunk(pio, pw, pt, c0, F, bias_t)
                c0 += P * F
    nc.compile()
    return nc


def _get_nc():
    if "nc" not in _NC_CACHE:
        _NC_CACHE["nc"] = _build_nc()
    return _NC_CACHE["nc"]


def _ref_decisions_cpu(verts, quads_subset):
    """Eager jnp on CPU — bit-exact mirror of the reference decision math."""
    import jax
    import jax.numpy as jnp

    cpu = jax.devices("cpu")[0]
    with jax.default_device(cpu):
        verts_j = jnp.asarray(verts)
        quads_j = jnp.asarray(quads_subset)
        EPS = 1e-12

        def normalize(x):
            n = jnp.linalg.norm(x, axis=-1, keepdims=True)
            return x / jnp.maximum(n, EPS)

        def max_cos(a, b, c):
            c1 = jnp.sum(normalize(b - a) * normalize(c - a), axis=-1)
            c2 = jnp.sum(normalize(c - b) * normalize(a - b), axis=-1)
            c3 = jnp.sum(normalize(a - c) * normalize(b - c), axis=-1)
            return jnp.maximum(jnp.maximum(c1, c2), c3)

        qv = verts_j[quads_j]
        v0, v1, v2, v3 = qv[:, 0], qv[:, 1], qv[:, 2], qv[:, 3]
        n1_c1 = jnp.cross(v1 - v0, v2 - v0)
        n2_c1 = jnp.cross(v2 - v0, v3 - v0)
        n1_c2 = jnp.cross(v1 - v0, v3 - v0)
        n2_c2 = jnp.cross(v2 - v1, v3 - v1)
        is_concave = (jnp.sum(n1_c1 * n2_c1, axis=-1) < 0) | \
                     (jnp.sum(n1_c2 * n2_c2, axis=-1) < 0)
        angles1 = jnp.maximum(max_cos(v0, v1, v3), max_cos(v1, v2, v3))
        angles2 = jnp.maximum(max_cos(v0, v1, v2), max_cos(v0, v2, v3))
        ge = angles1 >= angles2
        use_cfg1 = jnp.where(is_concave, ge, ~ge)
        group = jnp.where(is_concave,
                          jnp.where(ge, 0, 1),
                          jnp.where(ge, 3, 2)).astype(jnp.int32)
        return np.asarray(group), np.asarray(use_cfg1)


def _ensure_ntff_hook():
    """Register the axon NTFF profiling hook (the minimal agent image lacks
    antenv.axon_hooks, so trn_boot degrades silently; re-wire it here)."""
    import types

    if "antenv.axon_hooks" not in sys.modules:
        mod = types.ModuleType("antenv.axon_hooks")
        mod._hook = None
        mod.set_axon_ntff_profile_hook = lambda h: setattr(mod, "_hook", h)
        mod.get_axon_ntff_profile_hook = lambda: mod._hook
        sys.modules["antenv.axon_hooks"] = mod
        try:
            import antenv
            antenv.axon_hooks = mod
        except Exception:
            pass
    mod = sys.modules["antenv.axon_hooks"]
    if mod.get_axon_ntff_profile_hook() is None:
        try:
            if "/root/.axon_site" not in sys.path:
                sys.path.append("/root/.axon_site")
            from trn_agent_boot import trn_boot
            hook = trn_boot._ntff_profile_via_ctypes("/opt/axon/libaxon_pjrt.so")
            if hook is not None:
                mod.set_axon_ntff_profile_hook(hook)
        except Exception:
            pass


def kernel(verts, quads, profile=False):
    global LAST_EXEC_TIME_NS
    from concourse import bass_utils

    verts = np.ascontiguousarray(np.asarray(verts, dtype=np.float32))
    quads = np.ascontiguousarray(np.asarray(quads, dtype=np.int32))
    Q = quads.shape[0]
    assert Q == Q_TOTAL, f"kernel hardcoded for Q={Q_TOTAL}, got {Q}"

    # host gather (random access) + shard
    qv = verts[quads.ravel()].reshape(-1, 12)   # [Q, 12] float32

    in_maps = []
    pad = N_PAD - N_SHARD
    for c in range(NCORES):
        lo = c * N_SHARD
        hi = lo + N_SHARD
        q_sh = np.concatenate([quads[lo:hi], np.repeat(quads[hi - 1:hi], pad, 0)])
        v_sh = np.concatenate([qv[lo:hi], np.repeat(qv[hi - 1:hi], pad, 0)])
        in_maps.append({"qv": np.ascontiguousarray(v_sh),
                        "quads": np.ascontiguousarray(q_sh)})

    nc = _get_nc()
    res = None
    if profile:
        try:
            _ensure_ntff_hook()
            bass_utils.upload_artifacts = lambda tmpdir: tmpdir  # no bucket here
            res = bass_utils.run_bass_kernel_spmd(
                nc, in_maps, core_ids=list(range(NCORES)), trace=True)
            LAST_EXEC_TIME_NS = res.exec_time_ns
        except Exception as e:
            print(f"profiled run failed ({e!r}); falling back to untraced run")
            res = None
    if res is None:
        res = bass_utils.run_bass_kernel_spmd(
            nc, in_maps, core_ids=list(range(NCORES)))

    tri = np.concatenate([res.results[c]["tri"][:N_SHARD] for c in range(NCORES)])
    grp = np.concatenate([res.results[c]["grp"][:N_SHARD] for c in range(NCORES)])

    gi = grp.astype(np.int32)
    risk = gi >= 4
    group = gi & 3

    # resolve risk-flagged quads with the exact reference decision math (CPU)
    ridx = np.flatnonzero(risk)
    if len(ridx):
        g_fix, u_fix = _ref_decisions_cpu(verts, quads[ridx])
        group[ridx] = g_fix
        qr = quads[ridx]
        tri[ridx, 2] = np.where(u_fix, qr[:, 3], qr[:, 2])
        tri[ridx, 3] = np.where(u_fix, qr[:, 1], qr[:, 0])

    order = np.concatenate([np.flatnonzero(group == g) for g in range(4)])
    faces = tri[order].reshape(-1, 3)
    return verts, faces
